# revision 1
# baseline (speedup 1.0000x reference)
"""Trainium2 Bass kernel for nn_Aggregator (GNN message passing).

Computes, for each node n:
    out[n, :] = curr_emb[n, 0, :] + sum_k alpha[n, k] * msg[n, k, :]

Full shapes: curr_emb [16384, 32, 256] f32, alpha [16384, 32, 1] f32,
msg [16384, 32, 256] f32 -> out [16384, 256] f32.

Strategy: shard the node dimension N across 8 NeuronCores (2048 nodes each).
On each core, nodes ride the 128 SBUF partitions; per 128-node tile we run a
chain of fused multiply-add ops (scalar_tensor_tensor: acc = msg_k*alpha_k +
acc) on VectorE, seeded with curr_emb[:, 0, :].  msg streams through SBUF in
4 MiB contiguous DMAs; the kernel is HBM-bandwidth-bound (~68 MiB per core).

Only the needed slice curr_emb[:, 0, :] is shipped to the device.
"""

import numpy as np

N_FULL, K, D = 16384, 32, 256
N_CORES = 8
N_SHARD = N_FULL // N_CORES  # 2048
P = 128  # SBUF partitions


def _build_kernel_body(tc, out, curr0, alpha, msg, n_shard):
    import concourse.bass as bass
    from concourse import mybir

    nc = tc.nc
    ntiles = n_shard // P
    KD = K * D

    msg_t = msg.rearrange("(t p) k d -> t p (k d)", p=P)
    curr_t = curr0.rearrange("(t p) d -> t p d", p=P)
    alpha_t = alpha.rearrange("(t p) k -> t p k", p=P)
    out_t = out.rearrange("(t p) d -> t p d", p=P)

    with (
        tc.tile_pool(name="msg", bufs=3) as msg_pool,
        tc.tile_pool(name="small", bufs=4) as small_pool,
        tc.tile_pool(name="acc", bufs=4) as acc_pool,
    ):
        for t in range(ntiles):
            msg_tile = msg_pool.tile([P, KD], mybir.dt.float32, tag="msg")
            # 4 MiB contiguous load, split in 4 x 1 MiB chunks so compute can
            # start on the first chunk while later ones are in flight.
            for c in range(4):
                nc.sync.dma_start(
                    msg_tile[:, c * (KD // 4) : (c + 1) * (KD // 4)],
                    msg_t[t][:, c * (KD // 4) : (c + 1) * (KD // 4)],
                )
            alpha_tile = small_pool.tile([P, K], mybir.dt.float32, tag="alpha")
            nc.sync.dma_start(alpha_tile[:], alpha_t[t])
            curr_tile = small_pool.tile([P, D], mybir.dt.float32, tag="curr")
            nc.sync.dma_start(curr_tile[:], curr_t[t])

            acc = acc_pool.tile([P, D], mybir.dt.float32, tag="acc")
            for k in range(K):
                nc.vector.scalar_tensor_tensor(
                    out=acc[:],
                    in0=msg_tile[:, k * D : (k + 1) * D],
                    scalar=alpha_tile[:, k : k + 1],
                    in1=(curr_tile if k == 0 else acc)[:],
                    op0=mybir.AluOpType.mult,
                    op1=mybir.AluOpType.add,
                )
            nc.sync.dma_start(out_t[t], acc[:])


def build_nc(n_shard=N_SHARD):
    import concourse.tile as tile
    from concourse import bacc, mybir

    nc = bacc.Bacc(
        "TRN2",
        target_bir_lowering=False,
        debug=False,
        enable_asserts=False,
        num_devices=N_CORES,
    )
    curr0 = nc.dram_tensor(
        "curr0", (n_shard, D), mybir.dt.float32, kind="ExternalInput"
    ).ap()
    alpha = nc.dram_tensor(
        "alpha", (n_shard, K), mybir.dt.float32, kind="ExternalInput"
    ).ap()
    msg = nc.dram_tensor(
        "msg", (n_shard, K, D), mybir.dt.float32, kind="ExternalInput"
    ).ap()
    out = nc.dram_tensor(
        "out", (n_shard, D), mybir.dt.float32, kind="ExternalOutput"
    ).ap()

    with tile.TileContext(nc) as tc:
        _build_kernel_body(tc, out, curr0, alpha, msg, n_shard)
    nc.compile()
    return nc


_NC_CACHE = {}


def _get_nc():
    if "nc" not in _NC_CACHE:
        _NC_CACHE["nc"] = build_nc()
    return _NC_CACHE["nc"]


def make_in_maps(curr_emb, alpha, msg):
    curr_emb = np.asarray(curr_emb)
    alpha = np.asarray(alpha)
    msg = np.asarray(msg)
    in_maps = []
    for c in range(N_CORES):
        sl = slice(c * N_SHARD, (c + 1) * N_SHARD)
        in_maps.append(
            {
                "curr0": np.ascontiguousarray(curr_emb[sl, 0, :], dtype=np.float32),
                "alpha": np.ascontiguousarray(alpha[sl, :, 0], dtype=np.float32),
                "msg": np.ascontiguousarray(msg[sl], dtype=np.float32),
            }
        )
    return in_maps


def run(curr_emb, alpha, msg, trace=False):
    """Run on the 8 NeuronCores; returns (full output, BassKernelResults)."""
    from concourse import bass_utils

    nc = _get_nc()
    in_maps = make_in_maps(curr_emb, alpha, msg)
    res = bass_utils.run_bass_kernel_spmd(
        nc, in_maps, core_ids=list(range(N_CORES)), trace=trace
    )
    full = np.concatenate([r["out"] for r in res.results], axis=0)
    return full, res


def kernel(curr_emb, alpha, msg):
    return run(curr_emb, alpha, msg)[0]


# revision 12
# speedup vs baseline: 1.7702x; 1.7702x over previous
"""Trainium2 Bass kernel for nn_Aggregator (GNN message passing).

Computes, for each node n:
    out[n, :] = curr_emb[n, 0, :] + sum_k alpha[n, k] * msg[n, k, :]

Full shapes: curr_emb [16384, 32, 256] f32, alpha [16384, 32, 1] f32,
msg [16384, 32, 256] f32 -> out [16384, 256] f32.

Sharding: node dimension N split across 8 NeuronCores (2048 nodes each),
fully independent per-node work, no collectives.  Only the needed slice
curr_emb[:, 0, :] is shipped to the device.

Per-core algorithm (v2, TensorE reduction):
  Nodes are processed in groups of 4; a group's mailbox msg[4, 32, 256]
  occupies all 128 SBUF partitions as [128=(4 nodes x 32 slots), 256].
  The weighted K-sum for 4 nodes is ONE matmul with a block-diagonal
  stationary matrix lhsT[128, 4] holding alpha of node m in partition band
  [32m, 32m+32).  The block-diagonal tiles for all 512 groups are built
  once at kernel start from alpha (PE transpose + one broadcast multiply
  with a constant band mask), so the steady state is: stream msg via DMA,
  one matmul per group into PSUM (32 groups fill a [128, 256] PSUM tile =
  128 nodes), one VectorE add of curr_emb slot 0, DMA out.  The kernel is
  HBM-bandwidth-bound (~68 MiB per core).
"""

import numpy as np

N_FULL, K, D = 16384, 32, 256
N_CORES = 8
N_SHARD = N_FULL // N_CORES  # 2048
P = 128  # SBUF partitions
GRP = P // K  # 4 nodes per matmul group
GROUPS_PER_TILE = P // GRP  # 32 groups -> 128 nodes per PSUM batch


def _build_body_v2(tc, out, curr0, alpha, msg, n_shard):
    """TensorE reduction: out_tile = sum_k diag(alpha[:, k]) @ msg_k + curr.

    Per 128-node tile, 32 matmuls accumulate into one [128, 256] PSUM
    region.  The diagonal stationary for slot k is identity * alpha[:, k]
    (one 2x-mode tensor_scalar, alternating VectorE/ScalarE).  Matmuls run
    in float32r (same bits as f32; PE streams 1 row/cycle vs 4 for f32).
    """
    from concourse import masks, mybir

    nc = tc.nc
    ntiles = n_shard // P
    KD = K * D
    f32 = mybir.dt.float32
    f32r = mybir.dt.float32r

    msg_t = msg.rearrange("(t p) k d -> t p (k d)", p=P)
    curr_t = curr0.rearrange("(t p) d -> t p d", p=P)
    alpha_t = alpha.rearrange("(t p) k -> t p k", p=P)
    out_t = out.rearrange("(t p) d -> t p d", p=P)

    with (
        tc.tile_pool(name="const", bufs=1) as const_pool,
        tc.tile_pool(name="msg", bufs=3) as msg_pool,
        tc.tile_pool(name="small", bufs=4) as small_pool,
        tc.tile_pool(name="diag", bufs=6) as diag_pool,
        tc.tile_pool(name="psum", bufs=4, space="PSUM") as psum_pool,
    ):
        ident = const_pool.tile([P, P], f32, tag="ident")
        masks.make_identity(nc, ident[:])

        for t in range(ntiles):
            # float32r tile: gpsimd (SWDGE) DMA performs the f32 -> f32r
            # rounding cast the FP32r matmult verifier requires.
            msg_tile = msg_pool.tile([P, KD], f32r, tag="msg")
            for c in range(4):
                nc.gpsimd.dma_start(
                    msg_tile[:, c * (KD // 4) : (c + 1) * (KD // 4)],
                    msg_t[t][:, c * (KD // 4) : (c + 1) * (KD // 4)],
                )
            alpha_tile = small_pool.tile([P, K], f32, tag="alpha")
            nc.sync.dma_start(alpha_tile[:], alpha_t[t])
            curr_tile = small_pool.tile([P, D], f32, tag="curr")
            nc.sync.dma_start(curr_tile[:], curr_t[t])

            psum_tile = psum_pool.tile([P, D], f32, tag="ps")
            for k in range(K):
                diagk = diag_pool.tile([P, P], f32r, tag="diag")
                if k % 2 == 0:
                    nc.vector.tensor_scalar(
                        out=diagk[:],
                        in0=ident[:],
                        scalar1=alpha_tile[:, k : k + 1],
                        scalar2=None,
                        op0=mybir.AluOpType.mult,
                    )
                else:
                    nc.scalar.activation(
                        diagk[:],
                        ident[:],
                        mybir.ActivationFunctionType.Copy,
                        scale=alpha_tile[:, k : k + 1],
                    )
                nc.tensor.matmul(
                    psum_tile[:],
                    diagk[:],
                    msg_tile[:, k * D : (k + 1) * D],
                    start=(k == 0),
                    stop=(k == K - 1),
                )

            out_sb = small_pool.tile([P, D], f32, tag="osb")
            nc.vector.tensor_add(out_sb[:], psum_tile[:], curr_tile[:])
            nc.sync.dma_start(out_t[t], out_sb[:])


def _build_body_v1(tc, out, curr0, alpha, msg, n_shard):
    """VectorE-only scalar_tensor_tensor chain (fallback/reference)."""
    from concourse import mybir

    nc = tc.nc
    ntiles = n_shard // P
    KD = K * D

    msg_t = msg.rearrange("(t p) k d -> t p (k d)", p=P)
    curr_t = curr0.rearrange("(t p) d -> t p d", p=P)
    alpha_t = alpha.rearrange("(t p) k -> t p k", p=P)
    out_t = out.rearrange("(t p) d -> t p d", p=P)

    with (
        tc.tile_pool(name="msg", bufs=3) as msg_pool,
        tc.tile_pool(name="small", bufs=4) as small_pool,
        tc.tile_pool(name="acc", bufs=4) as acc_pool,
    ):
        for t in range(ntiles):
            msg_tile = msg_pool.tile([P, KD], mybir.dt.float32, tag="msg")
            for c in range(4):
                nc.sync.dma_start(
                    msg_tile[:, c * (KD // 4) : (c + 1) * (KD // 4)],
                    msg_t[t][:, c * (KD // 4) : (c + 1) * (KD // 4)],
                )
            alpha_tile = small_pool.tile([P, K], mybir.dt.float32, tag="alpha")
            nc.sync.dma_start(alpha_tile[:], alpha_t[t])
            curr_tile = small_pool.tile([P, D], mybir.dt.float32, tag="curr")
            nc.sync.dma_start(curr_tile[:], curr_t[t])

            acc = acc_pool.tile([P, D], mybir.dt.float32, tag="acc")
            for k in range(K):
                nc.vector.scalar_tensor_tensor(
                    out=acc[:],
                    in0=msg_tile[:, k * D : (k + 1) * D],
                    scalar=alpha_tile[:, k : k + 1],
                    in1=(curr_tile if k == 0 else acc)[:],
                    op0=mybir.AluOpType.mult,
                    op1=mybir.AluOpType.add,
                )
            nc.sync.dma_start(out_t[t], acc[:])


def build_nc(n_shard=N_SHARD, version=2):
    import concourse.tile as tile
    from concourse import bacc, mybir

    nc = bacc.Bacc(
        "TRN2",
        target_bir_lowering=False,
        debug=False,
        enable_asserts=False,
        num_devices=N_CORES,
    )
    curr0 = nc.dram_tensor(
        "curr0", (n_shard, D), mybir.dt.float32, kind="ExternalInput"
    ).ap()
    alpha = nc.dram_tensor(
        "alpha", (n_shard, K), mybir.dt.float32, kind="ExternalInput"
    ).ap()
    msg = nc.dram_tensor(
        "msg", (n_shard, K, D), mybir.dt.float32, kind="ExternalInput"
    ).ap()
    out = nc.dram_tensor(
        "out", (n_shard, D), mybir.dt.float32, kind="ExternalOutput"
    ).ap()

    body = _build_body_v2 if version == 2 else _build_body_v1
    with tile.TileContext(nc) as tc:
        body(tc, out, curr0, alpha, msg, n_shard)
    nc.compile()
    return nc


_NC_CACHE = {}


def _get_nc():
    if "nc" not in _NC_CACHE:
        _NC_CACHE["nc"] = build_nc()
    return _NC_CACHE["nc"]


def make_in_maps(curr_emb, alpha, msg):
    curr_emb = np.asarray(curr_emb)
    alpha = np.asarray(alpha)
    msg = np.asarray(msg)
    in_maps = []
    for c in range(N_CORES):
        sl = slice(c * N_SHARD, (c + 1) * N_SHARD)
        in_maps.append(
            {
                "curr0": np.ascontiguousarray(curr_emb[sl, 0, :], dtype=np.float32),
                "alpha": np.ascontiguousarray(alpha[sl, :, 0], dtype=np.float32),
                "msg": np.ascontiguousarray(msg[sl], dtype=np.float32),
            }
        )
    return in_maps


def run(curr_emb, alpha, msg, trace=False):
    """Run on the 8 NeuronCores; returns (full output, BassKernelResults)."""
    from concourse import bass_utils

    nc = _get_nc()
    in_maps = make_in_maps(curr_emb, alpha, msg)
    res = bass_utils.run_bass_kernel_spmd(
        nc, in_maps, core_ids=list(range(N_CORES)), trace=trace
    )
    full = np.concatenate([r["out"] for r in res.results], axis=0)
    return full, res


def kernel(curr_emb, alpha, msg):
    return run(curr_emb, alpha, msg)[0]
